# revision 5
# baseline (speedup 1.0000x reference)
"""CSPN 3x3 propagation step on 8 Trainium2 NeuronCores.

out[b,0,r,c] = sum_k aff[b,k,r,c] * patch_k(cur)[r,c], with the center tap
(k=4) taken from coarse_seg instead of cur_seg. Zero padding at image edges.

Sharding: pure data parallel over batch (16 images -> 2 per core), one SPMD
Bass program run on all 8 cores with per-core input slices.

Per-core algorithm (per 512x512 image): rows are packed PARTITION-MAJOR,
r = 4p + t  (partition p in 0..127, sub-row t in 0..3), so a +-1 row shift
stays inside the partition (a free-dim offset) for 3 of the 4 sub-rows.
The block-edge rows (r = 4p-1 and r = 4p+4) are covered by two extra
[128, 512] edge-plane loads (stride-4 row gather from HBM, 256 KB each).

  - No TensorEngine, no PSUM, no evacuation: the whole kernel is DMA
    (both HWDGE rings, 8 KB descriptors for affinity/coarse/out) plus
    elementwise multiply/add split across DVE and Pool (GpSimd).
  - Taps: product P_k = aff_k * shifted cur (dx = free-dim column offset
    into zero-padded tiles; dy = sub-row offset or edge plane), center tap
    k=4 multiplies coarse_seg. Tree-sum into the output tile, DMA store.
"""

import sys

import numpy as np

if "/opt/trn_rl_repo" not in sys.path:
    sys.path.insert(0, "/opt/trn_rl_repo")

B_PER_CORE = 2
N_CORES = 8
H = 512
W = 512
NBLK = 4  # sub-rows per partition
WPAD = W + 2  # zero column on each side

_compiled = None
_compiled_reps = {}


def _build_program(reps=1):
    """reps>1 unrolls the whole per-core computation `reps` times inside one
    NEFF — used only to measure kernel time through the dispatch noise."""
    import concourse.bacc as bacc
    import concourse.mybir as mybir
    import concourse.tile as tile

    fp32 = mybir.dt.float32

    nc = bacc.Bacc(
        "TRN2",
        target_bir_lowering=False,
        debug=False,
        enable_asserts=False,
        num_devices=N_CORES,
    )

    aff_d = nc.dram_tensor(
        "affinity", [B_PER_CORE, 9, H, W], fp32, kind="ExternalInput"
    ).ap()
    cur_d = nc.dram_tensor(
        "cur_seg", [B_PER_CORE, 1, H, W], fp32, kind="ExternalInput"
    ).ap()
    coa_d = nc.dram_tensor(
        "coarse_seg", [B_PER_CORE, 1, H, W], fp32, kind="ExternalInput"
    ).ap()
    out_d = nc.dram_tensor(
        "out", [B_PER_CORE, 1, H, W], fp32, kind="ExternalOutput"
    ).ap()

    with tile.TileContext(nc) as tc:
        with (
            tc.tile_pool(name="cur", bufs=2) as cur_pool,
            tc.tile_pool(name="edge", bufs=4) as edge_pool,
            tc.tile_pool(name="coa", bufs=2) as coa_pool,
            tc.tile_pool(name="aff", bufs=9) as aff_pool,
            tc.tile_pool(name="prod", bufs=7) as prod_pool,
        ):
            for b in [bb for _ in range(reps) for bb in range(B_PER_CORE)]:
                # DMA issue order per ring (transfers serialize per ring):
                #   ACT: tM, ak7, ak8, ak1, ak3, ak5, out[0:2]
                #   SP:  ak6, tD, ak0, ak2, tU, tC, ak4, out[2:4]
                # Critical tiles (tM, ak6) lead both rings; late consumers
                # (tC, ak4, ak5) trail.

                def _load_aff(k, ring):
                    ak = aff_pool.tile([128, NBLK, W], fp32, tag="aff")
                    ring.dma_start(
                        out=ak[:],
                        in_=aff_d[b, k].rearrange("(p t) c -> p t c", p=128),
                    )
                    return ak

                # --- cur tile [128, 4, 514]: [p, t, 1+c] = cur[4p+t, c] ---
                tM = cur_pool.tile([128, NBLK, WPAD], fp32, tag="cur")
                nc.vector.memset(tM[:, :, 0:1], 0.0)
                nc.vector.memset(tM[:, :, WPAD - 1 : WPAD], 0.0)
                cur_rows = cur_d[b, 0].rearrange("(p t) c -> p t c", p=128)
                nc.scalar.dma_start(out=tM[:, :, 1 : W + 1], in_=cur_rows)

                a6 = _load_aff(6, nc.sync)
                a7 = _load_aff(7, nc.scalar)

                # --- edge planes [128, 514]: tD[p] = cur[4p-1], tU[p] = cur[4p+4]
                tD = edge_pool.tile([128, WPAD], fp32, tag="ed")
                nc.gpsimd.memset(tD[:], 0.0)
                dn_rows = cur_d[b, 0][3 : H - 1].rearrange("(p t) c -> p t c", t=4)
                nc.sync.dma_start(out=tD[1:128, 1 : W + 1], in_=dn_rows[:, 0, :])

                a8 = _load_aff(8, nc.scalar)
                a0 = _load_aff(0, nc.sync)

                tU = edge_pool.tile([128, WPAD], fp32, tag="eu")
                nc.gpsimd.memset(tU[:], 0.0)
                up_rows = cur_d[b, 0][4:H].rearrange("(p t) c -> p t c", t=4)
                nc.sync.dma_start(out=tU[0:127, 1 : W + 1], in_=up_rows[:, 0, :])

                # dx column windows into the padded tiles
                def mwin(tlo, thi, dxi):
                    return tM[:, tlo:thi, dxi : dxi + W]

                # --- group dy=+1 (k=6,7,8): patch row r+1 = [p, t+1] or tU ---
                # interior (t=0..2) from tM, block-edge row (t=3) from tU
                P6 = prod_pool.tile([128, NBLK, W], fp32, tag="prod")
                nc.vector.tensor_mul(out=P6[:, 0:3, :], in0=a6[:, 0:3, :], in1=mwin(1, 4, 0))
                nc.gpsimd.tensor_mul(out=P6[:, 3, :], in0=a6[:, 3, :], in1=tU[:, 0:W])
                P7 = prod_pool.tile([128, NBLK, W], fp32, tag="prod")
                nc.vector.tensor_mul(out=P7[:, 0:3, :], in0=a7[:, 0:3, :], in1=mwin(1, 4, 1))
                nc.gpsimd.tensor_mul(out=P7[:, 3, :], in0=a7[:, 3, :], in1=tU[:, 1 : 1 + W])
                nc.vector.tensor_add(out=P6[:], in0=P6[:], in1=P7[:])
                a1 = _load_aff(1, nc.scalar)
                P8 = prod_pool.tile([128, NBLK, W], fp32, tag="prod")
                nc.gpsimd.tensor_mul(out=P8[:, 0:3, :], in0=a8[:, 0:3, :], in1=mwin(1, 4, 2))
                nc.vector.tensor_mul(out=P8[:, 3, :], in0=a8[:, 3, :], in1=tU[:, 2 : 2 + W])
                nc.gpsimd.tensor_add(out=P6[:], in0=P6[:], in1=P8[:])

                a2 = _load_aff(2, nc.sync)

                # --- group dy=-1 (k=0,1,2): patch row r-1 = [p, t-1] or tD ---
                P0 = prod_pool.tile([128, NBLK, W], fp32, tag="prod")
                nc.gpsimd.tensor_mul(out=P0[:, 1:4, :], in0=a0[:, 1:4, :], in1=mwin(0, 3, 0))
                nc.vector.tensor_mul(out=P0[:, 0, :], in0=a0[:, 0, :], in1=tD[:, 0:W])
                P1 = prod_pool.tile([128, NBLK, W], fp32, tag="prod")
                nc.gpsimd.tensor_mul(out=P1[:, 1:4, :], in0=a1[:, 1:4, :], in1=mwin(0, 3, 1))
                nc.vector.tensor_mul(out=P1[:, 0, :], in0=a1[:, 0, :], in1=tD[:, 1 : 1 + W])
                nc.vector.tensor_add(out=P0[:], in0=P0[:], in1=P1[:])
                a3 = _load_aff(3, nc.scalar)
                P2 = prod_pool.tile([128, NBLK, W], fp32, tag="prod")
                nc.gpsimd.tensor_mul(out=P2[:, 1:4, :], in0=a2[:, 1:4, :], in1=mwin(0, 3, 2))
                nc.vector.tensor_mul(out=P2[:, 0, :], in0=a2[:, 0, :], in1=tD[:, 2 : 2 + W])
                nc.gpsimd.tensor_add(out=P0[:], in0=P0[:], in1=P2[:])

                # --- coarse tile [128, 4, 512] (center tap, no shift) ---
                tC = coa_pool.tile([128, NBLK, W], fp32, tag="coa")
                nc.sync.dma_start(
                    out=tC[:], in_=coa_d[b, 0].rearrange("(p t) c -> p t c", p=128)
                )
                a4 = _load_aff(4, nc.sync)

                # --- group dy=0 (k=3,4,5): k=4 uses coarse ---
                p3 = prod_pool.tile([128, NBLK, W], fp32, tag="prod")
                nc.vector.tensor_mul(out=p3[:], in0=a3[:], in1=mwin(0, 4, 0))
                p4 = prod_pool.tile([128, NBLK, W], fp32, tag="prod")
                nc.gpsimd.tensor_mul(out=p4[:], in0=a4[:], in1=tC[:])
                nc.vector.tensor_add(out=p3[:], in0=p3[:], in1=p4[:])
                a5 = _load_aff(5, nc.scalar)
                p5 = prod_pool.tile([128, NBLK, W], fp32, tag="prod")
                nc.gpsimd.tensor_mul(out=p5[:], in0=a5[:], in1=mwin(0, 4, 2))
                nc.gpsimd.tensor_add(out=p3[:], in0=p3[:], in1=p5[:])

                # --- final sum + store, split in halves for drain overlap ---
                out_rows = out_d[b, 0].rearrange("(p t) c -> p t c", p=128)
                nc.vector.tensor_add(
                    out=P6[:, 0:2, :], in0=P6[:, 0:2, :], in1=P0[:, 0:2, :]
                )
                nc.gpsimd.tensor_add(
                    out=P6[:, 0:2, :], in0=P6[:, 0:2, :], in1=p3[:, 0:2, :]
                )
                nc.scalar.dma_start(out=out_rows[:, 0:2, :], in_=P6[:, 0:2, :])
                nc.gpsimd.tensor_add(
                    out=P6[:, 2:4, :], in0=P6[:, 2:4, :], in1=P0[:, 2:4, :]
                )
                nc.vector.tensor_add(
                    out=P6[:, 2:4, :], in0=P6[:, 2:4, :], in1=p3[:, 2:4, :]
                )
                nc.sync.dma_start(out=out_rows[:, 2:4, :], in_=P6[:, 2:4, :])

    nc.compile()
    return nc


def _get_program(reps=1):
    global _compiled
    if reps != 1:
        if reps not in _compiled_reps:
            _compiled_reps[reps] = _build_program(reps)
        return _compiled_reps[reps]
    if _compiled is None:
        _compiled = _build_program()
    return _compiled


def _in_maps(affinity, cur_seg, coarse_seg):
    maps = []
    for j in range(N_CORES):
        s = slice(j * B_PER_CORE, (j + 1) * B_PER_CORE)
        maps.append(
            {
                "affinity": np.ascontiguousarray(affinity[s]),
                "cur_seg": np.ascontiguousarray(cur_seg[s]),
                "coarse_seg": np.ascontiguousarray(coarse_seg[s]),
            }
        )
    return maps


def kernel(affinity, cur_seg, coarse_seg, i=None, **_unused):
    from concourse.bass_utils import run_bass_kernel_spmd

    nc = _get_program()

    affinity = np.ascontiguousarray(affinity, dtype=np.float32)
    cur_seg = np.ascontiguousarray(cur_seg, dtype=np.float32)
    coarse_seg = np.ascontiguousarray(coarse_seg, dtype=np.float32)

    res = run_bass_kernel_spmd(
        nc, _in_maps(affinity, cur_seg, coarse_seg), core_ids=list(range(N_CORES))
    )
    out = np.concatenate([r["out"] for r in res.results], axis=0)
    return out
